# revision 24
# baseline (speedup 1.0000x reference)
"""Cross-attention kernel for Trainium2 (8 NeuronCores, SPMD).

Problem: B=4, Nq=1024, Nk=2048, D=512, 8 heads x 64 head-dim, fp32,
full-tensor bias added to scores before softmax.

Sharding: (batch, head-half) -> 8 shards, one per core. Each core computes
4 heads over the full 1024 queries of its batch and emits a PARTIAL output
projection (its 256 inner dims of Wo); the host adds the two partials per
batch. This halves the K/V projection work per core versus query-sharding
(K/V no longer computed redundantly) at the cost of a fp16 partial-sum
gather on the host.

Device layout: attention tensors kept transposed (feature/key dim on
partitions) so every matmul contraction lands on the partition axis:
  QT[d, q] = (SCALE*Wq_hh) @ xT       KT[d, k] = Wk_hh @ ctxT
  V[k, i]  = ctxT.T @ Wv_hh.T
  ST[k, q] = KT_h.T @ QT_h            (two heads of a pair in PE row groups
                                       0-1/2-3, concurrent)
  E = exp(ST) * exp(biasT - 4)        (ACT exp; DVE multiply against a
                                       step-0 broadcast of the host-side
                                       exp(bias-4).T tile)
  out2T[i(+1), q] = [V_h | 1].T @ E   (ones column gives softmax row-sums)
  OT = out2T[0:64] * recip(sum)       (DVE sums->SBUF, rank-2 selector
                                       matmul broadcasts per-query factors
                                       across the pair's partitions, fast
                                       approx reciprocal, one multiply)
  yT_part[d, q] = Wo_hh @ OT + bo/2   (bo enters as a rank-1 matmul; ACT
                                       evacuates fp16 for the store)
The inner loop runs 64 units (pair, q-half, chunk) software-pipelined two
ahead (scores lead exp/mul/AV), with K/Q prefetch for the next pair and
the V stream as TensorE fillers during the first block.
"""

import numpy as np
import concourse.bass as bass
import concourse.bacc as bacc
import concourse.mybir as mybir
import concourse.tile as tile
from concourse import bass_utils

HEADS = 8
HPC = 4           # heads per core
DH = 64
D = 512
IN2 = HPC * DH    # 256 inner dims per core
NQ = 1024         # full queries per core
QH = 512          # query half (matmul moving width)
NK = 2048
KC = NK // 128    # 16 key chunks
SCALE = DH ** -0.5
BSHIFT = 4.0

F32 = mybir.dt.float32
F16 = mybir.dt.float16
AF = mybir.ActivationFunctionType


def _bcast2(ap, n):
    """[128, F] -> [128, n, F] with a step-0 middle dim."""
    return bass.AP(ap.tensor, ap.offset, [ap.ap[0], [0, n], ap.ap[1]])


def _build_nc():
    nc = bacc.Bacc("TRN2", target_bir_lowering=False, debug=False)

    xT_d = nc.dram_tensor("xT", [D, NQ], F16, kind="ExternalInput")
    ctxT_d = nc.dram_tensor("ctxT", [D, NK], F16, kind="ExternalInput")
    expB_d = nc.dram_tensor("expB", [NK, NQ], F16, kind="ExternalInput")
    wqT_d = nc.dram_tensor("wqT", [D, IN2], F16, kind="ExternalInput")
    wkT_d = nc.dram_tensor("wkT", [D, IN2], F16, kind="ExternalInput")
    wvT_d = nc.dram_tensor("wvT", [D, IN2], F16, kind="ExternalInput")
    woT_d = nc.dram_tensor("woT", [IN2, D], F16, kind="ExternalInput")
    bo_d = nc.dram_tensor("bo", [1, D], F16, kind="ExternalInput")
    yT_d = nc.dram_tensor("yT", [D, NQ], F16, kind="ExternalOutput")

    with tile.TileContext(nc) as tc, nc.allow_low_precision(
            reason="fp16 matmul operands, fp32 accumulation"):
        with (
            tc.tile_pool(name="const", bufs=1) as const,
            tc.tile_pool(name="main", bufs=1) as main,
            tc.tile_pool(name="work", bufs=6) as work,
            tc.tile_pool(name="norm", bufs=3) as norm,
            tc.tile_pool(name="ctxp", bufs=1) as ctxp,
        ):
            wq = [const.tile([128, IN2], F16, name=f"wq{i}", tag=f"wq{i}") for i in range(4)]
            wk = [const.tile([128, IN2], F16, name=f"wk{i}", tag=f"wk{i}") for i in range(4)]
            wv = [const.tile([128, IN2], F16, name=f"wv{i}", tag=f"wv{i}") for i in range(4)]
            wo = [const.tile([128, D], F16, name=f"wo{i}", tag=f"wo{i}") for i in range(2)]
            boro = const.tile([1, D], F16, name="boro", tag="boro")
            onesF = const.tile([128, 1], F32, name="onesF", tag="onesF")
            nc.vector.memset(onesF, 1.0)
            onesq = const.tile([1, NQ], F16, name="onesq", tag="onesq")
            nc.vector.tensor_copy(onesq, onesF[0:1, 0:1].broadcast_to([1, NQ]))
            e2 = const.tile([33, 128], F16, name="e2", tag="e2")
            nc.vector.memset(e2, 0.0)
            nc.vector.memset(e2[0:1, 0:DH], 1.0)
            nc.vector.memset(e2[32:33, DH:128], 1.0)
            ss2 = const.tile([33, QH], F16, name="ss2", tag="ss2")
            nc.vector.memset(ss2, 0.0)

            ctx = [ctxp.tile([128, NK], F16, name=f"ctx{i}", tag=f"ctx{i}") for i in range(4)]
            xts = [ctxp.tile([128, NQ], F16, name=f"xts{i}", tag=f"xts{i}") for i in range(4)]
            # sync queue: wk, full-tile ctx (big DMAs sustain the best
            # rate), then the exp(bias) stream, then tail-only weights
            for i in range(4):
                nc.sync.dma_start(out=wk[i], in_=wkT_d[i * 128:(i + 1) * 128, :])
            for nt in range(2):
                nsl = slice(nt * 1024, (nt + 1) * 1024)
                for i in range(4):
                    nc.sync.dma_start(out=ctx[i][:, nsl],
                                      in_=ctxT_d[i * 128:(i + 1) * 128, nsl])
            # gpsimd queue: x + Wq for the early Q proj, then Wv
            for i in range(4):
                nc.gpsimd.dma_start(out=xts[i], in_=xT_d[i * 128:(i + 1) * 128, :])
            for i in range(4):
                nc.gpsimd.dma_start(out=wq[i], in_=wqT_d[i * 128:(i + 1) * 128, :])
            for i in range(4):
                nc.gpsimd.dma_start(out=wv[i], in_=wvT_d[i * 128:(i + 1) * 128, :])

            KT = [main.tile([128, NK], F16, name=f"KT{i}", tag=f"KT{i}") for i in range(2)]
            QT = [main.tile([128, NQ], F16, name=f"QT{i}", tag=f"QT{i}") for i in range(2)]
            OT = [main.tile([128, NQ], F16, name=f"OT{i}", tag=f"OT{i}") for i in range(2)]
            Vo = [main.tile([128, HPC, DH + 1], F16, name=f"Vo{c}", tag=f"Vo{c}")
                  for c in range(KC)]
            eB = [main.tile([128, NQ], F16, name=f"eB{c}", tag=f"eB{c}") for c in range(KC)]
            for c in range(KC):
                nc.vector.tensor_copy(
                    Vo[c][:, :, DH], onesF[:, 0:1].broadcast_to([128, HPC]))
            for c in range(KC):
                nc.sync.dma_start(out=eB[c], in_=expB_d[c * 128:(c + 1) * 128, :])
            for i in range(2):
                nc.sync.dma_start(out=wo[i], in_=woT_d[i * 128:(i + 1) * 128, :])
            nc.sync.dma_start(out=boro, in_=bo_d[:, :])

            def k_proj_group(psA, mi, nt):
                msl = slice(mi * 128, (mi + 1) * 128)
                nsl = slice(nt * 512, (nt + 1) * 512)
                ps = psA.tile([128, 512], F32, name="proj", tag="proj")
                for ki in range(4):
                    nc.tensor.matmul(
                        ps, wk[ki][:, msl], ctx[ki][:, nsl],
                        start=(ki == 0), stop=(ki == 3))
                nc.vector.tensor_copy(KT[mi][:, nsl], ps)

            def v_proj_group(psA, c, on_act=False):
                csl = slice(c * 128, (c + 1) * 128)
                ps = psA.tile([128, IN2], F32, name="vproj", tag="proj")
                for ki in range(4):
                    nc.tensor.matmul(
                        ps, ctx[ki][:, csl], wv[ki],
                        start=(ki == 0), stop=(ki == 3))
                src = ps.rearrange("p (h d) -> p h d", h=HPC)
                if on_act:
                    nc.scalar.copy(Vo[c][:, :, 0:DH], src)
                else:
                    nc.vector.tensor_copy(Vo[c][:, :, 0:DH], src)

            def q_proj_group(psA, mi):
                msl = slice(mi * 128, (mi + 1) * 128)
                for qh in range(2):
                    qsl = slice(qh * QH, (qh + 1) * QH)
                    ps = psA.tile([128, QH], F32, name="qproj", tag="proj")
                    for ki in range(4):
                        nc.tensor.matmul(
                            ps, wq[ki][:, msl], xts[ki][:, qsl],
                            start=(ki == 0), stop=(ki == 3))
                    nc.vector.tensor_copy(QT[mi][:, qsl], ps)

            # ---- upfront: just enough to start block (0,0) ----
            with tc.tile_pool(name="psA0", bufs=2, space="PSUM") as psA0:
                k_proj_group(psA0, 0, 0)
                k_proj_group(psA0, 0, 1)
                q_proj_group(psA0, 0)
                for c in range(2):
                    v_proj_group(psA0, c)

            # ---- attention: 64 units (pair, q-half, chunk), scores two
            # units ahead of the exp/mul/AV stage ----
            with (
                tc.tile_pool(name="psS", bufs=2, space="PSUM") as psS,
                tc.tile_pool(name="psO", bufs=2, space="PSUM") as psO,
                tc.tile_pool(name="psA", bufs=2, space="PSUM") as psA,
            ):
                lo, hi = slice(0, DH), slice(DH, 128)

                def out_chain(mi, qh, on_act):
                    # one (mi, q-half) slice of the partial output
                    # projection: rank-1 bo + two ki accumulations in a
                    # single-bank PSUM tile, evac, store
                    msl = slice(mi * 128, (mi + 1) * 128)
                    qsl = slice(qh * QH, (qh + 1) * QH)
                    ps = psA.tile([128, QH], F32, name="oc", tag="proj")
                    nc.tensor.matmul(
                        ps, boro[:, msl], onesq[:, qsl], start=True, stop=False)
                    for ki in range(2):
                        nc.tensor.matmul(
                            ps, wo[ki][:, msl], OT[ki][:, qsl],
                            start=False, stop=(ki == 1))
                    ysb = norm.tile([128, QH], F16, name="ysb", tag="ysb")
                    if on_act:
                        nc.scalar.copy(ysb, ps)
                    else:
                        nc.vector.tensor_copy(ysb, ps)
                    if (mi + qh) % 2 == 0:
                        nc.sync.dma_start(out=yT_d[msl, qsl], in_=ysb)
                    else:
                        nc.gpsimd.dma_start(out=yT_d[msl, qsl], in_=ysb)

                def fillers(hp, qh, c):
                    if hp == 1 and qh == 1:
                        # q-half-0 output chains ride the last block (their
                        # OT halves completed with norm(1,0))
                        if c in (2, 5, 8, 11):
                            out_chain({2: 0, 5: 1, 8: 2, 11: 3}[c], 0,
                                      on_act=False)
                    if hp == 0 and qh == 0:
                        if c == 0:
                            k_proj_group(psA, 0, 2)
                        elif c == 1:
                            k_proj_group(psA, 0, 3)
                        if c <= 13:
                            v_proj_group(psA, c + 2, on_act=(c % 3 == 2))
                        if c == 6:
                            k_proj_group(psA, 1, 0)
                        elif c == 10:
                            k_proj_group(psA, 1, 1)
                    elif hp == 0 and qh == 1:
                        if c == 0:
                            k_proj_group(psA, 1, 2)
                        elif c == 4:
                            k_proj_group(psA, 1, 3)
                        elif c == 8:
                            q_proj_group(psA, 1)

                def unit(g):
                    hp, r = divmod(g, 2 * KC)
                    qh, c = divmod(r, KC)
                    return hp, qh, c

                s_tiles, o2t = {}, {}
                for g in range(64 + 2):
                    if g < 64:
                        hp, qh, c = unit(g)
                        qsl = slice(qh * QH, (qh + 1) * QH)
                        csl = slice(c * 128, (c + 1) * 128)
                        s = psS.tile([128, 2, QH], F32, name="s", tag="s")
                        s_tiles[g] = s
                        nc.tensor.matmul(
                            s[:, 0, :], KT[hp][lo, csl], QT[hp][lo, qsl],
                            start=True, stop=True)
                        nc.tensor.matmul(
                            s[:, 1, :], KT[hp][hi, csl], QT[hp][hi, qsl],
                            start=True, stop=True)
                    if g >= 2:
                        hp, qh, c = unit(g - 2)
                        qsl = slice(qh * QH, (qh + 1) * QH)
                        h0, h1 = 2 * hp, 2 * hp + 1
                        if c == 0:
                            o2t[(hp, qh)] = (
                                psO.tile([DH + 1, QH], F32, name="o2a", tag="o2"),
                                psO.tile([DH + 1, QH], F32, name="o2b", tag="o2"))
                        o2a, o2b = o2t[(hp, qh)]
                        s = s_tiles.pop(g - 2)
                        e1 = work.tile([128, 2, QH], F16, name="e1", tag="e1")
                        nc.scalar.activation(e1, s, AF.Exp)
                        et = work.tile([128, 2, QH], F16, name="et", tag="et")
                        nc.vector.tensor_mul(et, e1, _bcast2(eB[c][:, qsl], 2))
                        nc.tensor.matmul(
                            o2a, Vo[c][:, h0, :], et[:, 0, :],
                            start=(c == 0), stop=(c == KC - 1))
                        nc.tensor.matmul(
                            o2b, Vo[c][:, h1, :], et[:, 1, :],
                            start=(c == 0), stop=(c == KC - 1))
                        fillers(hp, qh, c)
                        if c == KC - 1:
                            oUp = norm.tile([128, QH], F16, name="oUp", tag="oUp")
                            nc.vector.tensor_copy(oUp[0:DH, :], o2a[0:DH, :])
                            nc.vector.tensor_copy(oUp[DH:128, :], o2b[0:DH, :])
                            nc.vector.tensor_copy(ss2[0:1, :], o2a[DH:DH + 1, :])
                            nc.vector.tensor_copy(ss2[32:33, :], o2b[DH:DH + 1, :])
                            rb = psA.tile([128, QH], F32, name="rb", tag="proj")
                            nc.tensor.matmul(rb, e2, ss2, start=True, stop=True)
                            rrb = norm.tile([128, QH], F32, name="rrb", tag="rrb")
                            nc.vector.reciprocal_approx_fast(out=rrb, in_=rb)
                            nc.vector.tensor_mul(OT[hp][:, qsl], oUp, rrb)

                # drain: the q-half-1 output chains (need OT from the
                # final norm); evac on ACT, which is idle by now
                for mi in range(4):
                    out_chain(mi, 1, on_act=True)

    nc.compile()
    return nc


_NC_CACHE = {}


def _get_nc():
    if "nc" not in _NC_CACHE:
        _NC_CACHE["nc"] = _build_nc()
    return _NC_CACHE["nc"]


def make_in_maps(x, context, bias, Wq, Wk, Wv, Wo, bo):
    x = np.asarray(x, dtype=np.float32)
    context = np.asarray(context, dtype=np.float32)
    bias = np.asarray(bias, dtype=np.float32)
    Wq = np.asarray(Wq); Wk = np.asarray(Wk); Wv = np.asarray(Wv)
    Wo = np.asarray(Wo)
    # half of bo on each core so the host-side partial add reconstructs it
    bo2 = np.ascontiguousarray(
        (np.asarray(bo) * 0.5).reshape(1, D)).astype(np.float16)

    in_maps = []
    for core in range(8):
        b, hh = core // 2, core % 2
        hsl = slice(hh * IN2, (hh + 1) * IN2)
        in_maps.append({
            "xT": np.ascontiguousarray(x[b].T).astype(np.float16),
            "ctxT": np.ascontiguousarray(context[b].T).astype(np.float16),
            "expB": np.ascontiguousarray(
                np.exp(bias[b] - BSHIFT).T).astype(np.float16),
            "wqT": np.ascontiguousarray((Wq[hsl] * SCALE).T).astype(np.float16),
            "wkT": np.ascontiguousarray(Wk[hsl].T).astype(np.float16),
            "wvT": np.ascontiguousarray(Wv[hsl].T).astype(np.float16),
            "woT": np.ascontiguousarray(Wo[:, hsl].T).astype(np.float16),
            "bo": bo2,
        })
    return in_maps


def kernel(x, context, bias, Wq, Wk, Wv, Wo, bo):
    nc = _get_nc()
    in_maps = make_in_maps(x, context, bias, Wq, Wk, Wv, Wo, bo)
    res = bass_utils.run_bass_kernel_spmd(
        nc, in_maps, core_ids=list(range(8)), trace=False)

    out = np.empty((4, NQ, D), dtype=np.float32)
    for b in range(4):
        pa = res.results[2 * b]["yT"].astype(np.float32)
        pb = res.results[2 * b + 1]["yT"].astype(np.float32)
        out[b] = (pa + pb).T
    return out


# revision 26
# speedup vs baseline: 1.1839x; 1.1839x over previous
"""Cross-attention kernel for Trainium2 (8 NeuronCores, SPMD).

Problem: B=4, Nq=1024, Nk=2048, D=512, 8 heads x 64 head-dim, fp32,
full-tensor bias added to scores before softmax.

Sharding: (batch, head-half) -> 8 shards, one per core. Each core computes
4 heads over the full 1024 queries of its batch and emits a PARTIAL output
projection (its 256 inner dims of Wo); the host adds the two partials per
batch. This halves the K/V projection work per core versus query-sharding
(K/V no longer computed redundantly) at the cost of a fp16 partial-sum
gather on the host.

Device layout: attention tensors kept transposed (feature/key dim on
partitions) so every matmul contraction lands on the partition axis:
  QT[d, q] = (SCALE*Wq_hh) @ xT       KT[d, k] = Wk_hh @ ctxT
  V[k, i]  = ctxT.T @ Wv_hh.T
  ST[k, q] = KT_h.T @ QT_h            (two heads of a pair in PE row groups
                                       0-1/2-3, concurrent)
  E = exp(ST) * exp(biasT - 4)        (ACT exp; DVE multiply against a
                                       step-0 broadcast of the host-side
                                       exp(bias-4).T tile)
  out2T[i(+1), q] = [V_h | 1].T @ E   (ones column gives softmax row-sums)
  OT = out2T[0:64] * recip(sum)       (DVE sums->SBUF, rank-2 selector
                                       matmul broadcasts per-query factors
                                       across the pair's partitions, fast
                                       approx reciprocal, one multiply)
  yT_part[d, q] = Wo_hh @ OT + bo/2   (bo enters as a rank-1 matmul; ACT
                                       evacuates fp16 for the store)
The inner loop runs 64 units (pair, q-half, chunk) software-pipelined two
ahead (scores lead exp/mul/AV), with K/Q prefetch for the next pair and
the V stream as TensorE fillers during the first block.
"""

import numpy as np
import concourse.bass as bass
import concourse.bacc as bacc
import concourse.mybir as mybir
import concourse.tile as tile
from concourse import bass_utils

HEADS = 8
HPC = 4           # heads per core
DH = 64
D = 512
IN2 = HPC * DH    # 256 inner dims per core
NQ = 1024         # full queries per core
QH = 512          # query half (matmul moving width)
NK = 2048
KC = NK // 128    # 16 key chunks
SCALE = DH ** -0.5
BSHIFT = 4.0

F32 = mybir.dt.float32
F16 = mybir.dt.float16
AF = mybir.ActivationFunctionType


def _bcast2(ap, n):
    """[128, F] -> [128, n, F] with a step-0 middle dim."""
    return bass.AP(ap.tensor, ap.offset, [ap.ap[0], [0, n], ap.ap[1]])


def _build_nc():
    nc = bacc.Bacc("TRN2", target_bir_lowering=False, debug=False)

    xT_d = nc.dram_tensor("xT", [D, NQ], F16, kind="ExternalInput")
    ctxT_d = nc.dram_tensor("ctxT", [D, NK], F16, kind="ExternalInput")
    expB_d = nc.dram_tensor("expB", [NK, NQ], F16, kind="ExternalInput")
    wqT_d = nc.dram_tensor("wqT", [D, IN2], F16, kind="ExternalInput")
    wkT_d = nc.dram_tensor("wkT", [D, IN2], F16, kind="ExternalInput")
    wvT_d = nc.dram_tensor("wvT", [D, IN2], F16, kind="ExternalInput")
    woT_d = nc.dram_tensor("woT", [IN2, D], F16, kind="ExternalInput")
    bo_d = nc.dram_tensor("bo", [1, D], F16, kind="ExternalInput")
    yT_d = nc.dram_tensor("yT", [D, NQ], F16, kind="ExternalOutput")

    with tile.TileContext(nc) as tc, nc.allow_low_precision(
            reason="fp16 matmul operands, fp32 accumulation"):
        with (
            tc.tile_pool(name="const", bufs=1) as const,
            tc.tile_pool(name="main", bufs=1) as main,
            tc.tile_pool(name="work", bufs=6) as work,
            tc.tile_pool(name="norm", bufs=3) as norm,
            tc.tile_pool(name="ctxp", bufs=1) as ctxp,
        ):
            wq = [const.tile([128, IN2], F16, name=f"wq{i}", tag=f"wq{i}") for i in range(4)]
            wk = [const.tile([128, IN2], F16, name=f"wk{i}", tag=f"wk{i}") for i in range(4)]
            wv = [const.tile([128, IN2], F16, name=f"wv{i}", tag=f"wv{i}") for i in range(4)]
            wo = [const.tile([128, D], F16, name=f"wo{i}", tag=f"wo{i}") for i in range(2)]
            boro = const.tile([1, D], F16, name="boro", tag="boro")
            onesF = const.tile([128, 1], F32, name="onesF", tag="onesF")
            nc.vector.memset(onesF, 1.0)
            onesq = const.tile([1, NQ], F16, name="onesq", tag="onesq")
            nc.vector.tensor_copy(onesq, onesF[0:1, 0:1].broadcast_to([1, NQ]))
            e2 = const.tile([33, 128], F16, name="e2", tag="e2")
            nc.vector.memset(e2, 0.0)
            nc.vector.memset(e2[0:1, 0:DH], 1.0)
            nc.vector.memset(e2[32:33, DH:128], 1.0)
            ss2 = const.tile([33, QH], F16, name="ss2", tag="ss2")
            nc.vector.memset(ss2, 0.0)

            ctx = [ctxp.tile([128, NK], F16, name=f"ctx{i}", tag=f"ctx{i}") for i in range(4)]
            xts = [ctxp.tile([128, NQ], F16, name=f"xts{i}", tag=f"xts{i}") for i in range(4)]
            # sync queue: wk, full-tile ctx (big DMAs sustain the best
            # rate), then the exp(bias) stream, then tail-only weights
            for i in range(4):
                nc.sync.dma_start(out=wk[i], in_=wkT_d[i * 128:(i + 1) * 128, :])
            for i in range(4):
                nc.sync.dma_start(out=ctx[i], in_=ctxT_d[i * 128:(i + 1) * 128, :])
            # gpsimd queue: x + Wq for the early Q proj, then Wv
            for i in range(4):
                nc.gpsimd.dma_start(out=xts[i], in_=xT_d[i * 128:(i + 1) * 128, :])
            for i in range(4):
                nc.gpsimd.dma_start(out=wq[i], in_=wqT_d[i * 128:(i + 1) * 128, :])
            for i in range(4):
                nc.gpsimd.dma_start(out=wv[i], in_=wvT_d[i * 128:(i + 1) * 128, :])

            KT = [main.tile([128, NK], F16, name=f"KT{i}", tag=f"KT{i}") for i in range(2)]
            QT = [main.tile([128, NQ], F16, name=f"QT{i}", tag=f"QT{i}") for i in range(2)]
            OT = [main.tile([128, NQ], F16, name=f"OT{i}", tag=f"OT{i}") for i in range(2)]
            Vo = [main.tile([128, HPC, DH + 1], F16, name=f"Vo{c}", tag=f"Vo{c}")
                  for c in range(KC)]
            eB = [main.tile([128, NQ], F16, name=f"eB{c}", tag=f"eB{c}") for c in range(KC)]
            for c in range(KC):
                nc.vector.tensor_copy(
                    Vo[c][:, :, DH], onesF[:, 0:1].broadcast_to([128, HPC]))
            for c in range(KC):
                q = nc.sync if c % 2 == 0 else nc.gpsimd
                q.dma_start(out=eB[c], in_=expB_d[c * 128:(c + 1) * 128, :])
            for i in range(2):
                nc.sync.dma_start(out=wo[i], in_=woT_d[i * 128:(i + 1) * 128, :])
            nc.sync.dma_start(out=boro, in_=bo_d[:, :])

            def k_proj_group(psA, mi, nt):
                msl = slice(mi * 128, (mi + 1) * 128)
                nsl = slice(nt * 512, (nt + 1) * 512)
                ps = psA.tile([128, 512], F32, name="proj", tag="proj")
                for ki in range(4):
                    nc.tensor.matmul(
                        ps, wk[ki][:, msl], ctx[ki][:, nsl],
                        start=(ki == 0), stop=(ki == 3))
                nc.vector.tensor_copy(KT[mi][:, nsl], ps)

            def v_proj_group(psA, c, on_act=False):
                csl = slice(c * 128, (c + 1) * 128)
                ps = psA.tile([128, IN2], F32, name="vproj", tag="proj")
                for ki in range(4):
                    nc.tensor.matmul(
                        ps, ctx[ki][:, csl], wv[ki],
                        start=(ki == 0), stop=(ki == 3))
                src = ps.rearrange("p (h d) -> p h d", h=HPC)
                if on_act:
                    nc.scalar.copy(Vo[c][:, :, 0:DH], src)
                else:
                    nc.vector.tensor_copy(Vo[c][:, :, 0:DH], src)

            def q_proj_group(psA, mi):
                msl = slice(mi * 128, (mi + 1) * 128)
                for qh in range(2):
                    qsl = slice(qh * QH, (qh + 1) * QH)
                    ps = psA.tile([128, QH], F32, name="qproj", tag="proj")
                    for ki in range(4):
                        nc.tensor.matmul(
                            ps, wq[ki][:, msl], xts[ki][:, qsl],
                            start=(ki == 0), stop=(ki == 3))
                    nc.vector.tensor_copy(QT[mi][:, qsl], ps)

            # ---- upfront: just enough to start block (0,0) ----
            with tc.tile_pool(name="psA0", bufs=2, space="PSUM") as psA0:
                k_proj_group(psA0, 0, 0)
                k_proj_group(psA0, 0, 1)
                q_proj_group(psA0, 0)
                for c in range(2):
                    v_proj_group(psA0, c)

            # ---- attention: 64 units (pair, q-half, chunk), scores two
            # units ahead of the exp/mul/AV stage ----
            with (
                tc.tile_pool(name="psS", bufs=2, space="PSUM") as psS,
                tc.tile_pool(name="psO", bufs=2, space="PSUM") as psO,
                tc.tile_pool(name="psA", bufs=2, space="PSUM") as psA,
            ):
                lo, hi = slice(0, DH), slice(DH, 128)

                def out_chain(mi, qh, on_act):
                    # one (mi, q-half) slice of the partial output
                    # projection: rank-1 bo + two ki accumulations in a
                    # single-bank PSUM tile, evac, store
                    msl = slice(mi * 128, (mi + 1) * 128)
                    qsl = slice(qh * QH, (qh + 1) * QH)
                    ps = psA.tile([128, QH], F32, name="oc", tag="proj")
                    nc.tensor.matmul(
                        ps, boro[:, msl], onesq[:, qsl], start=True, stop=False)
                    for ki in range(2):
                        nc.tensor.matmul(
                            ps, wo[ki][:, msl], OT[ki][:, qsl],
                            start=False, stop=(ki == 1))
                    ysb = norm.tile([128, QH], F16, name="ysb", tag="ysb")
                    if on_act:
                        nc.scalar.copy(ysb, ps)
                    else:
                        nc.vector.tensor_copy(ysb, ps)
                    if (mi + qh) % 2 == 0:
                        nc.sync.dma_start(out=yT_d[msl, qsl], in_=ysb)
                    else:
                        nc.gpsimd.dma_start(out=yT_d[msl, qsl], in_=ysb)

                def fillers(hp, qh, c):
                    if hp == 1 and qh == 1:
                        # q-half-0 output chains ride the last block (their
                        # OT halves completed with norm(1,0))
                        if c in (2, 5, 8, 11):
                            out_chain({2: 0, 5: 1, 8: 2, 11: 3}[c], 0,
                                      on_act=False)
                    if hp == 0 and qh == 0:
                        if c == 0:
                            k_proj_group(psA, 0, 2)
                        elif c == 1:
                            k_proj_group(psA, 0, 3)
                        if c <= 13:
                            v_proj_group(psA, c + 2, on_act=(c % 3 == 2))
                        if c == 6:
                            k_proj_group(psA, 1, 0)
                        elif c == 10:
                            k_proj_group(psA, 1, 1)
                    elif hp == 0 and qh == 1:
                        if c == 0:
                            k_proj_group(psA, 1, 2)
                        elif c == 4:
                            k_proj_group(psA, 1, 3)
                        elif c == 8:
                            q_proj_group(psA, 1)

                def unit(g):
                    hp, r = divmod(g, 2 * KC)
                    qh, c = divmod(r, KC)
                    return hp, qh, c

                s_tiles, o2t = {}, {}
                for g in range(64 + 2):
                    if g < 64:
                        hp, qh, c = unit(g)
                        qsl = slice(qh * QH, (qh + 1) * QH)
                        csl = slice(c * 128, (c + 1) * 128)
                        s = psS.tile([128, 2, QH], F32, name="s", tag="s")
                        s_tiles[g] = s
                        nc.tensor.matmul(
                            s[:, 0, :], KT[hp][lo, csl], QT[hp][lo, qsl],
                            start=True, stop=True)
                        nc.tensor.matmul(
                            s[:, 1, :], KT[hp][hi, csl], QT[hp][hi, qsl],
                            start=True, stop=True)
                    if g >= 2:
                        hp, qh, c = unit(g - 2)
                        qsl = slice(qh * QH, (qh + 1) * QH)
                        h0, h1 = 2 * hp, 2 * hp + 1
                        if c == 0:
                            o2t[(hp, qh)] = (
                                psO.tile([DH + 1, QH], F32, name="o2a", tag="o2"),
                                psO.tile([DH + 1, QH], F32, name="o2b", tag="o2"))
                        o2a, o2b = o2t[(hp, qh)]
                        s = s_tiles.pop(g - 2)
                        e1 = work.tile([128, 2, QH], F16, name="e1", tag="e1")
                        nc.scalar.activation(e1, s, AF.Exp)
                        et = work.tile([128, 2, QH], F16, name="et", tag="et")
                        nc.vector.tensor_mul(et, e1, _bcast2(eB[c][:, qsl], 2))
                        nc.tensor.matmul(
                            o2a, Vo[c][:, h0, :], et[:, 0, :],
                            start=(c == 0), stop=(c == KC - 1))
                        nc.tensor.matmul(
                            o2b, Vo[c][:, h1, :], et[:, 1, :],
                            start=(c == 0), stop=(c == KC - 1))
                        fillers(hp, qh, c)
                        if c == KC - 1:
                            oUp = norm.tile([128, QH], F16, name="oUp", tag="oUp")
                            nc.vector.tensor_copy(oUp[0:DH, :], o2a[0:DH, :])
                            nc.vector.tensor_copy(oUp[DH:128, :], o2b[0:DH, :])
                            nc.vector.tensor_copy(ss2[0:1, :], o2a[DH:DH + 1, :])
                            nc.vector.tensor_copy(ss2[32:33, :], o2b[DH:DH + 1, :])
                            rb = psA.tile([128, QH], F32, name="rb", tag="proj")
                            nc.tensor.matmul(rb, e2, ss2, start=True, stop=True)
                            rrb = norm.tile([128, QH], F32, name="rrb", tag="rrb")
                            nc.vector.reciprocal_approx_fast(out=rrb, in_=rb)
                            nc.vector.tensor_mul(OT[hp][:, qsl], oUp, rrb)

                # drain: the q-half-1 output chains (need OT from the
                # final norm); evac on ACT, which is idle by now
                for mi in range(4):
                    out_chain(mi, 1, on_act=True)

    nc.compile()
    return nc


_NC_CACHE = {}


def _get_nc():
    if "nc" not in _NC_CACHE:
        _NC_CACHE["nc"] = _build_nc()
    return _NC_CACHE["nc"]


def make_in_maps(x, context, bias, Wq, Wk, Wv, Wo, bo):
    x = np.asarray(x, dtype=np.float32)
    context = np.asarray(context, dtype=np.float32)
    bias = np.asarray(bias, dtype=np.float32)
    Wq = np.asarray(Wq); Wk = np.asarray(Wk); Wv = np.asarray(Wv)
    Wo = np.asarray(Wo)
    # half of bo on each core so the host-side partial add reconstructs it
    bo2 = np.ascontiguousarray(
        (np.asarray(bo) * 0.5).reshape(1, D)).astype(np.float16)

    in_maps = []
    for core in range(8):
        b, hh = core // 2, core % 2
        hsl = slice(hh * IN2, (hh + 1) * IN2)
        in_maps.append({
            "xT": np.ascontiguousarray(x[b].T).astype(np.float16),
            "ctxT": np.ascontiguousarray(context[b].T).astype(np.float16),
            "expB": np.ascontiguousarray(
                np.exp(bias[b] - BSHIFT).T).astype(np.float16),
            "wqT": np.ascontiguousarray((Wq[hsl] * SCALE).T).astype(np.float16),
            "wkT": np.ascontiguousarray(Wk[hsl].T).astype(np.float16),
            "wvT": np.ascontiguousarray(Wv[hsl].T).astype(np.float16),
            "woT": np.ascontiguousarray(Wo[:, hsl].T).astype(np.float16),
            "bo": bo2,
        })
    return in_maps


def kernel(x, context, bias, Wq, Wk, Wv, Wo, bo):
    nc = _get_nc()
    in_maps = make_in_maps(x, context, bias, Wq, Wk, Wv, Wo, bo)
    res = bass_utils.run_bass_kernel_spmd(
        nc, in_maps, core_ids=list(range(8)), trace=False)

    out = np.empty((4, NQ, D), dtype=np.float32)
    for b in range(4):
        pa = res.results[2 * b]["yT"].astype(np.float32)
        pb = res.results[2 * b + 1]["yT"].astype(np.float32)
        out[b] = (pa + pb).T
    return out


# revision 30
# speedup vs baseline: 1.2125x; 1.0242x over previous
"""Cross-attention kernel for Trainium2 (8 NeuronCores, SPMD).

Problem: B=4, Nq=1024, Nk=2048, D=512, 8 heads x 64 head-dim, fp32,
full-tensor bias added to scores before softmax.

Sharding: (batch, head-half) -> 8 shards, one per core. Each core computes
4 heads over the full 1024 queries of its batch and emits a PARTIAL output
projection (its 256 inner dims of Wo); the host adds the two partials per
batch. This halves the K/V projection work per core versus query-sharding
(K/V no longer computed redundantly) at the cost of a fp16 partial-sum
gather on the host.

Device layout: attention tensors kept transposed (feature/key dim on
partitions) so every matmul contraction lands on the partition axis:
  QT[d, q] = (SCALE*Wq_hh) @ xT       KT[d, k] = Wk_hh @ ctxT
  V[k, i]  = ctxT.T @ Wv_hh.T
  ST[k, q] = KT_h.T @ QT_h            (two heads of a pair in PE row groups
                                       0-1/2-3, concurrent)
  E = exp(ST) * exp(biasT - 4)        (ACT exp; DVE multiply against a
                                       step-0 broadcast of the host-side
                                       exp(bias-4).T tile)
  out2T[i(+1), q] = [V_h | 1].T @ E   (ones column gives softmax row-sums)
  OT = out2T[0:64] * recip(sum)       (DVE sums->SBUF, rank-2 selector
                                       matmul broadcasts per-query factors
                                       across the pair's partitions, fast
                                       approx reciprocal, one multiply)
  yT_part[d, q] = Wo_hh @ OT + bo/2   (bo enters as a rank-1 matmul; ACT
                                       evacuates fp16 for the store)
The inner loop runs 64 units (pair, q-half, chunk) software-pipelined two
ahead (scores lead exp/mul/AV), with K/Q prefetch for the next pair and
the V stream as TensorE fillers during the first block.
"""

import numpy as np
import concourse.bass as bass
import concourse.bacc as bacc
import concourse.mybir as mybir
import concourse.tile as tile
from concourse import bass_utils

HEADS = 8
HPC = 4           # heads per core
DH = 64
D = 512
IN2 = HPC * DH    # 256 inner dims per core
NQ = 1024         # full queries per core
QH = 512          # query half (matmul moving width)
NK = 2048
KC = NK // 128    # 16 key chunks
SCALE = DH ** -0.5
BSHIFT = 4.0

F32 = mybir.dt.float32
F16 = mybir.dt.float16
AF = mybir.ActivationFunctionType


def _bcast2(ap, n):
    """[128, F] -> [128, n, F] with a step-0 middle dim."""
    return bass.AP(ap.tensor, ap.offset, [ap.ap[0], [0, n], ap.ap[1]])


def _build_nc():
    nc = bacc.Bacc("TRN2", target_bir_lowering=False, debug=False)

    xT_d = nc.dram_tensor("xT", [D, NQ], F16, kind="ExternalInput")
    ctxT_d = nc.dram_tensor("ctxT", [D, NK], F16, kind="ExternalInput")
    expB_d = nc.dram_tensor("expB", [NK, NQ], F16, kind="ExternalInput")
    wqT_d = nc.dram_tensor("wqT", [D, IN2], F16, kind="ExternalInput")
    wkT_d = nc.dram_tensor("wkT", [D, IN2], F16, kind="ExternalInput")
    wvT_d = nc.dram_tensor("wvT", [D, IN2], F16, kind="ExternalInput")
    woT_d = nc.dram_tensor("woT", [IN2, D], F16, kind="ExternalInput")
    bo_d = nc.dram_tensor("bo", [1, D], F16, kind="ExternalInput")
    yT_d = nc.dram_tensor("yT", [D, NQ], F16, kind="ExternalOutput")

    with tile.TileContext(nc) as tc, nc.allow_low_precision(
            reason="fp16 matmul operands, fp32 accumulation"):
        with (
            tc.tile_pool(name="const", bufs=1) as const,
            tc.tile_pool(name="main", bufs=1) as main,
            tc.tile_pool(name="work", bufs=8) as work,
            tc.tile_pool(name="norm", bufs=3) as norm,
            tc.tile_pool(name="ctxp", bufs=1) as ctxp,
        ):
            wq = [const.tile([128, IN2], F16, name=f"wq{i}", tag=f"wq{i}") for i in range(4)]
            wk = [const.tile([128, IN2], F16, name=f"wk{i}", tag=f"wk{i}") for i in range(4)]
            wv = [const.tile([128, IN2], F16, name=f"wv{i}", tag=f"wv{i}") for i in range(4)]
            wo = [const.tile([128, D], F16, name=f"wo{i}", tag=f"wo{i}") for i in range(2)]
            boro = const.tile([1, D], F16, name="boro", tag="boro")
            onesF = const.tile([128, 1], F32, name="onesF", tag="onesF")
            nc.vector.memset(onesF, 1.0)
            onesq = const.tile([1, NQ], F16, name="onesq", tag="onesq")
            nc.vector.tensor_copy(onesq, onesF[0:1, 0:1].broadcast_to([1, NQ]))
            e2 = const.tile([33, 128], F16, name="e2", tag="e2")
            nc.vector.memset(e2, 0.0)
            nc.vector.memset(e2[0:1, 0:DH], 1.0)
            nc.vector.memset(e2[32:33, DH:128], 1.0)
            ss2 = const.tile([33, QH], F16, name="ss2", tag="ss2")
            nc.vector.memset(ss2, 0.0)

            ctx = [ctxp.tile([128, NK], F16, name=f"ctx{i}", tag=f"ctx{i}") for i in range(4)]
            xts = [ctxp.tile([128, NQ], F16, name=f"xts{i}", tag=f"xts{i}") for i in range(4)]
            # sync queue: wk, full-tile ctx (big DMAs sustain the best
            # rate), then the exp(bias) stream, then tail-only weights
            for i in range(4):
                nc.sync.dma_start(out=ctx[i], in_=ctxT_d[i * 128:(i + 1) * 128, :])
            # gpsimd queue: x + Wq for the early Q proj, then Wk, Wv
            for i in range(4):
                nc.gpsimd.dma_start(out=xts[i], in_=xT_d[i * 128:(i + 1) * 128, :])
            for i in range(4):
                nc.gpsimd.dma_start(out=wq[i], in_=wqT_d[i * 128:(i + 1) * 128, :])
            for i in range(4):
                nc.gpsimd.dma_start(out=wk[i], in_=wkT_d[i * 128:(i + 1) * 128, :])
            for i in range(4):
                nc.gpsimd.dma_start(out=wv[i], in_=wvT_d[i * 128:(i + 1) * 128, :])

            KT = [main.tile([128, NK], F16, name=f"KT{i}", tag=f"KT{i}") for i in range(2)]
            QT = [main.tile([128, NQ], F16, name=f"QT{i}", tag=f"QT{i}") for i in range(2)]
            OT = [main.tile([128, NQ], F16, name=f"OT{i}", tag=f"OT{i}") for i in range(2)]
            Vo = [main.tile([128, HPC, DH + 1], F16, name=f"Vo{c}", tag=f"Vo{c}")
                  for c in range(KC)]
            eB = [main.tile([128, NQ], F16, name=f"eB{c}", tag=f"eB{c}") for c in range(KC)]
            for c in range(KC):
                nc.vector.tensor_copy(
                    Vo[c][:, :, DH], onesF[:, 0:1].broadcast_to([128, HPC]))
            for c in range(KC):
                q = nc.sync if c % 2 == 0 else nc.gpsimd
                q.dma_start(out=eB[c], in_=expB_d[c * 128:(c + 1) * 128, :])
            for i in range(2):
                nc.sync.dma_start(out=wo[i], in_=woT_d[i * 128:(i + 1) * 128, :])
            nc.sync.dma_start(out=boro, in_=bo_d[:, :])

            def k_proj_group(psA, mi, nt):
                msl = slice(mi * 128, (mi + 1) * 128)
                nsl = slice(nt * 512, (nt + 1) * 512)
                ps = psA.tile([128, 512], F32, name="proj", tag="proj")
                for ki in range(4):
                    nc.tensor.matmul(
                        ps, wk[ki][:, msl], ctx[ki][:, nsl],
                        start=(ki == 0), stop=(ki == 3))
                nc.vector.tensor_copy(KT[mi][:, nsl], ps)

            def v_proj_group(psA, c, on_act=False):
                csl = slice(c * 128, (c + 1) * 128)
                ps = psA.tile([128, IN2], F32, name="vproj", tag="proj")
                for ki in range(4):
                    nc.tensor.matmul(
                        ps, ctx[ki][:, csl], wv[ki],
                        start=(ki == 0), stop=(ki == 3))
                src = ps.rearrange("p (h d) -> p h d", h=HPC)
                if on_act:
                    nc.scalar.copy(Vo[c][:, :, 0:DH], src)
                else:
                    nc.vector.tensor_copy(Vo[c][:, :, 0:DH], src)

            def q_proj_group(psA, mi):
                msl = slice(mi * 128, (mi + 1) * 128)
                for qh in range(2):
                    qsl = slice(qh * QH, (qh + 1) * QH)
                    ps = psA.tile([128, QH], F32, name="qproj", tag="proj")
                    for ki in range(4):
                        nc.tensor.matmul(
                            ps, wq[ki][:, msl], xts[ki][:, qsl],
                            start=(ki == 0), stop=(ki == 3))
                    nc.vector.tensor_copy(QT[mi][:, qsl], ps)

            # ---- upfront: just enough to start block (0,0) ----
            with tc.tile_pool(name="psA0", bufs=2, space="PSUM") as psA0:
                q_proj_group(psA0, 0)
                k_proj_group(psA0, 0, 0)
                k_proj_group(psA0, 0, 1)
                for c in range(2):
                    v_proj_group(psA0, c)

            # ---- attention: 64 units (pair, q-half, chunk), scores two
            # units ahead of the exp/mul/AV stage ----
            with (
                tc.tile_pool(name="psS", bufs=2, space="PSUM") as psS,
                tc.tile_pool(name="psO", bufs=2, space="PSUM") as psO,
                tc.tile_pool(name="psA", bufs=2, space="PSUM") as psA,
            ):
                lo, hi = slice(0, DH), slice(DH, 128)

                def out_chain(mi, qh, on_act):
                    # one (mi, q-half) slice of the partial output
                    # projection: rank-1 bo + two ki accumulations in a
                    # single-bank PSUM tile, evac, store
                    msl = slice(mi * 128, (mi + 1) * 128)
                    qsl = slice(qh * QH, (qh + 1) * QH)
                    ps = psA.tile([128, QH], F32, name="oc", tag="proj")
                    nc.tensor.matmul(
                        ps, boro[:, msl], onesq[:, qsl], start=True, stop=False)
                    for ki in range(2):
                        nc.tensor.matmul(
                            ps, wo[ki][:, msl], OT[ki][:, qsl],
                            start=False, stop=(ki == 1))
                    ysb = norm.tile([128, QH], F16, name="ysb", tag="ysb")
                    if on_act:
                        nc.scalar.copy(ysb, ps)
                    else:
                        nc.vector.tensor_copy(ysb, ps)
                    if (mi + qh) % 2 == 0:
                        nc.sync.dma_start(out=yT_d[msl, qsl], in_=ysb)
                    else:
                        nc.gpsimd.dma_start(out=yT_d[msl, qsl], in_=ysb)

                def fillers(hp, qh, c):
                    if hp == 1 and qh == 1:
                        # q-half-0 output chains ride the last block (their
                        # OT halves completed with norm(1,0))
                        if c in (2, 5, 8, 11):
                            out_chain({2: 0, 5: 1, 8: 2, 11: 3}[c], 0,
                                      on_act=False)
                    if hp == 0 and qh == 0:
                        if c == 0:
                            k_proj_group(psA, 0, 2)
                        elif c == 1:
                            k_proj_group(psA, 0, 3)
                        if c <= 13:
                            v_proj_group(psA, c + 2, on_act=(c % 3 == 2))
                        if c == 6:
                            k_proj_group(psA, 1, 0)
                        elif c == 10:
                            k_proj_group(psA, 1, 1)
                    elif hp == 0 and qh == 1:
                        if c == 0:
                            k_proj_group(psA, 1, 2)
                        elif c == 4:
                            k_proj_group(psA, 1, 3)
                        elif c == 8:
                            q_proj_group(psA, 1)

                def unit(g):
                    hp, r = divmod(g, 2 * KC)
                    qh, c = divmod(r, KC)
                    return hp, qh, c

                s_tiles, o2t = {}, {}
                for g in range(64 + 2):
                    if g < 64:
                        hp, qh, c = unit(g)
                        qsl = slice(qh * QH, (qh + 1) * QH)
                        csl = slice(c * 128, (c + 1) * 128)
                        s = psS.tile([128, 2, QH], F32, name="s", tag="s")
                        s_tiles[g] = s
                        nc.tensor.matmul(
                            s[:, 0, :], KT[hp][lo, csl], QT[hp][lo, qsl],
                            start=True, stop=True)
                        nc.tensor.matmul(
                            s[:, 1, :], KT[hp][hi, csl], QT[hp][hi, qsl],
                            start=True, stop=True)
                    if g >= 2:
                        hp, qh, c = unit(g - 2)
                        qsl = slice(qh * QH, (qh + 1) * QH)
                        h0, h1 = 2 * hp, 2 * hp + 1
                        if c == 0:
                            o2t[(hp, qh)] = (
                                psO.tile([DH + 1, QH], F32, name="o2a", tag="o2"),
                                psO.tile([DH + 1, QH], F32, name="o2b", tag="o2"))
                        o2a, o2b = o2t[(hp, qh)]
                        s = s_tiles.pop(g - 2)
                        e1 = work.tile([128, 2, QH], F16, name="e1", tag="e1")
                        nc.scalar.activation(e1, s, AF.Exp)
                        et = work.tile([128, 2, QH], F16, name="et", tag="et")
                        nc.vector.tensor_mul(et, e1, _bcast2(eB[c][:, qsl], 2))
                        nc.tensor.matmul(
                            o2a, Vo[c][:, h0, :], et[:, 0, :],
                            start=(c == 0), stop=(c == KC - 1))
                        nc.tensor.matmul(
                            o2b, Vo[c][:, h1, :], et[:, 1, :],
                            start=(c == 0), stop=(c == KC - 1))
                        fillers(hp, qh, c)
                        if c == KC - 1:
                            oUp = norm.tile([128, QH], F16, name="oUp", tag="oUp")
                            if (hp, qh) == (1, 1):
                                # tail: ACT is idle, split the evacuation
                                nc.scalar.copy(oUp[0:DH, :], o2a[0:DH, :])
                            else:
                                nc.vector.tensor_copy(oUp[0:DH, :], o2a[0:DH, :])
                            nc.vector.tensor_copy(oUp[DH:128, :], o2b[0:DH, :])
                            nc.vector.tensor_copy(ss2[0:1, :], o2a[DH:DH + 1, :])
                            nc.vector.tensor_copy(ss2[32:33, :], o2b[DH:DH + 1, :])
                            rb = psA.tile([128, QH], F32, name="rb", tag="proj")
                            nc.tensor.matmul(rb, e2, ss2, start=True, stop=True)
                            rrb = norm.tile([128, QH], F32, name="rrb", tag="rrb")
                            nc.vector.reciprocal_approx_fast(out=rrb, in_=rb)
                            nc.vector.tensor_mul(OT[hp][:, qsl], oUp, rrb)

                # drain: the q-half-1 output chains (need OT from the
                # final norm); evac on ACT, which is idle by now
                for mi in range(4):
                    out_chain(mi, 1, on_act=True)

    nc.compile()
    return nc


_NC_CACHE = {}


def _get_nc():
    if "nc" not in _NC_CACHE:
        _NC_CACHE["nc"] = _build_nc()
    return _NC_CACHE["nc"]


def make_in_maps(x, context, bias, Wq, Wk, Wv, Wo, bo):
    x = np.asarray(x, dtype=np.float32)
    context = np.asarray(context, dtype=np.float32)
    bias = np.asarray(bias, dtype=np.float32)
    Wq = np.asarray(Wq); Wk = np.asarray(Wk); Wv = np.asarray(Wv)
    Wo = np.asarray(Wo)
    # half of bo on each core so the host-side partial add reconstructs it
    bo2 = np.ascontiguousarray(
        (np.asarray(bo) * 0.5).reshape(1, D)).astype(np.float16)

    in_maps = []
    for core in range(8):
        b, hh = core // 2, core % 2
        hsl = slice(hh * IN2, (hh + 1) * IN2)
        in_maps.append({
            "xT": np.ascontiguousarray(x[b].T).astype(np.float16),
            "ctxT": np.ascontiguousarray(context[b].T).astype(np.float16),
            "expB": np.ascontiguousarray(
                np.exp(bias[b] - BSHIFT).T).astype(np.float16),
            "wqT": np.ascontiguousarray((Wq[hsl] * SCALE).T).astype(np.float16),
            "wkT": np.ascontiguousarray(Wk[hsl].T).astype(np.float16),
            "wvT": np.ascontiguousarray(Wv[hsl].T).astype(np.float16),
            "woT": np.ascontiguousarray(Wo[:, hsl].T).astype(np.float16),
            "bo": bo2,
        })
    return in_maps


def kernel(x, context, bias, Wq, Wk, Wv, Wo, bo):
    nc = _get_nc()
    in_maps = make_in_maps(x, context, bias, Wq, Wk, Wv, Wo, bo)
    res = bass_utils.run_bass_kernel_spmd(
        nc, in_maps, core_ids=list(range(8)), trace=False)

    out = np.empty((4, NQ, D), dtype=np.float32)
    for b in range(4):
        pa = res.results[2 * b]["yT"].astype(np.float32)
        pb = res.results[2 * b + 1]["yT"].astype(np.float32)
        out[b] = (pa + pb).T
    return out
